# revision 12
# baseline (speedup 1.0000x reference)
"""Trainium2 Bass kernel for nn_AttentionPooling (topk_masking).

Computation per graph b (B=32 graphs, batch-sharded over 8 NeuronCores):
  alpha_pre = clip(x @ w, +-60); alpha = exp(alpha_pre)*mask / (sum + eps)
  x_out = x * alpha[:, None]
  drop the N_remove = round(0.2*N_nodes) valid nodes with smallest alpha:
    rank[n] computed by sign-counting: acc[n] = sum_m sign(key[m]-key[n])
    (key = alpha for valid nodes, 9.0 for invalid) => acc = 1023 - 2*rank,
    keep <=> rank >= N_remove <=> acc <= 1023 - 2*N_remove  (and valid)
  A_out = A * outer(keep, keep);  mask_new = keep;  N_pooled = sum(keep)

Device mapping per core (4 graphs):
  VectorE: matvec (tensor_tensor_reduce), x scaling, A *= mm, small [128,8] ops
  ScalarE: exp (+row-accumulate), Sign compare tiles (vs key broadcast, with
           per-partition -key bias and free-dim accumulate = rank in one pass)
  TensorE: partition-sum reductions (ones-matmul), [128,8] transposes,
           keep outer-products (bf16, exact for 0/1 masks)
  DMA:     A/x streaming; key-row broadcast built by SBUF->SBUF gather+bcast
"""

import numpy as np

import concourse.bass as bass
import concourse.bacc as bacc
import concourse.mybir as mybir
import concourse.tile as tile
from concourse import bass_utils

B, N, C = 32, 1024, 256
P = 128
NCH = N // P              # 8 chunks of 128 nodes
NCORES = 8
GPC = B // NCORES         # graphs per core
QQ = 2                    # A chunks per DMA (2 -> 1MB tiles, 2MB loads)
TOPK_RATIO = 0.8
CLAMP = 60.0
EPS = 1e-7

f32 = mybir.dt.float32
bf16 = mybir.dt.bfloat16
u8 = mybir.dt.uint8
i32 = mybir.dt.int32
AF = mybir.ActivationFunctionType
OP = mybir.AluOpType
AX = mybir.AxisListType


def build_program(gpc=GPC, nch=NCH):
    from contextlib import ExitStack

    nc = bacc.Bacc("TRN2", target_bir_lowering=False, debug=False)

    x_d = nc.dram_tensor("x_in", [gpc, N, C], f32, kind="ExternalInput").ap()
    a_d = nc.dram_tensor("a_in", [gpc, N, N], f32, kind="ExternalInput").ap()
    mp_d = nc.dram_tensor("mask_p", [gpc, P, nch], f32, kind="ExternalInput").ap()
    mr_d = nc.dram_tensor("mask_r", [gpc, N], f32, kind="ExternalInput").ap()
    thr_d = nc.dram_tensor("thr", [gpc], f32, kind="ExternalInput").ap()
    w_d = nc.dram_tensor("w", [C], f32, kind="ExternalInput").ap()
    id_d = nc.dram_tensor("ident", [P, P], f32, kind="ExternalInput").ap()

    xo_d = nc.dram_tensor("x_out", [gpc, N, C], f32, kind="ExternalOutput").ap()
    ao_d = nc.dram_tensor("a_out", [gpc, N, N], f32, kind="ExternalOutput").ap()
    mn_d = nc.dram_tensor("mask_new", [gpc, N], u8, kind="ExternalOutput").ap()
    al_d = nc.dram_tensor("alpha_out", [gpc, N], f32, kind="ExternalOutput").ap()
    npl_d = nc.dram_tensor("n_pooled", [gpc], i32, kind="ExternalOutput").ap()
    ks_d = nc.dram_tensor("key_scratch", [gpc, N], f32).ap()

    with tile.TileContext(nc) as tc, ExitStack() as ctx:
        consts = ctx.enter_context(tc.tile_pool(name="consts", bufs=1))
        xpool = ctx.enter_context(tc.tile_pool(name="xpool", bufs=2))
        small = ctx.enter_context(tc.tile_pool(name="small", bufs=2))
        rows = ctx.enter_context(tc.tile_pool(name="rows", bufs=gpc))
        kbp = ctx.enter_context(tc.tile_pool(name="kbp", bufs=2))
        signp = ctx.enter_context(tc.tile_pool(name="signp", bufs=2))
        apool = ctx.enter_context(tc.tile_pool(name="apool", bufs=3))
        scratch = ctx.enter_context(tc.tile_pool(name="scratch", bufs=2))
        psmm = ctx.enter_context(tc.tile_pool(name="psmm", bufs=2, space="PSUM"))
        psmisc = ctx.enter_context(tc.tile_pool(name="psmisc", bufs=3, space="PSUM"))

        # constants
        w8 = consts.tile([P, C], f32)
        nc.gpsimd.dma_start(w8, bass.AP(w_d.tensor, 0, [[0, P], [1, C]]))
        ident = consts.tile([P, P], f32)
        nc.sync.dma_start(ident, id_d)
        thr_bc = consts.tile([P, gpc], f32)
        nc.gpsimd.dma_start(thr_bc, bass.AP(thr_d.tensor, 0, [[0, P], [1, gpc]]))
        ones_col = consts.tile([P, 1], f32)
        nc.vector.memset(ones_col, 1.0)
        ones_row = consts.tile([1, P], f32)
        nc.vector.memset(ones_row, 1.0)
        npl_row = consts.tile([1, gpc], i32)

        keep_rows = []

        # ---------------- phase 1: per-graph alpha / top-k mask ----------
        for g in range(gpc):
            x_t = xpool.tile([P, nch, C], f32, tag="x_t")
            nc.sync.dma_start(x_t, x_d[g].rearrange("(j p) c -> p j c", p=P))
            mp = small.tile([P, nch], f32, tag="mp")
            nc.sync.dma_start(mp, mp_d[g])
            mr8 = small.tile([nch, P], f32, tag="mr8")
            nc.sync.dma_start(mr8, mr_d[g].rearrange("(j q) -> j q", j=nch))

            # matvec: pre[:, j] = sum_c x[:, j, c] * w[c]
            pre = small.tile([P, nch], f32, tag="pre")
            prod = scratch.tile([P, nch, C], f32, tag="prod")
            nc.vector.tensor_mul(
                prod, x_t, w8.unsqueeze(1).to_broadcast((P, nch, C))
            )
            nc.vector.tensor_reduce(pre, prod, axis=AX.X, op=OP.add)
            nc.vector.tensor_scalar(pre, pre, CLAMP, None, op0=OP.min)
            nc.vector.tensor_scalar(pre, pre, -CLAMP, None, op0=OP.max)

            # exp(pre - 1000*(1-mask)) -> exact 0 on invalid nodes
            shift = small.tile([P, nch], f32, tag="shift")
            nc.vector.tensor_scalar(shift, mp, 1000.0, 1000.0, op0=OP.mult,
                                    op1=OP.subtract)
            pre2 = small.tile([P, nch], f32, tag="pre2")
            nc.vector.tensor_add(pre2, pre, shift)
            e_t = small.tile([P, nch], f32, tag="e_t")
            esum = small.tile([P, 1], f32, tag="esum")
            nc.scalar.activation(e_t, pre2, AF.Exp, accum_out=esum)

            # S = sum(esum) over partitions; Sinv = 1/(S+eps)
            s_ps = psmisc.tile([1, 1], f32, tag="misc")
            nc.tensor.matmul(s_ps, lhsT=ones_col, rhs=esum, start=True, stop=True)
            seps = small.tile([1, 1], f32, tag="seps")
            nc.vector.tensor_scalar(seps, s_ps, EPS, None, op0=OP.add)
            sinv = small.tile([1, 1], f32, tag="sinv")
            nc.vector.reciprocal(sinv, seps)
            sinv_ps = psmisc.tile([P, 1], f32, tag="misc")
            nc.tensor.matmul(sinv_ps, lhsT=ones_row, rhs=sinv, start=True, stop=True)
            sinv_col = small.tile([P, 1], f32, tag="sinv_col")
            nc.scalar.copy(sinv_col, sinv_ps)

            alpha_p = small.tile([P, nch], f32, tag="alpha_p")
            nc.vector.tensor_scalar(alpha_p, e_t, sinv_col, None, op0=OP.mult)

            # x_out = x * alpha (in place), store
            for j in range(nch):
                nc.vector.tensor_scalar(x_t[:, j, :], x_t[:, j, :],
                                        alpha_p[:, j : j + 1], None, op0=OP.mult)
            nc.gpsimd.dma_start(xo_d[g].rearrange("(j p) c -> p j c", p=P), x_t)

            # key = alpha + 9*(1-mask); neg for use as activation bias
            inv9 = small.tile([P, nch], f32, tag="inv9")
            nc.vector.tensor_scalar(inv9, mp, -9.0, 9.0, op0=OP.mult, op1=OP.add)
            key_p = small.tile([P, nch], f32, tag="key_p")
            nc.vector.tensor_add(key_p, alpha_p, inv9)
            nkey_p = small.tile([P, nch], f32, tag="nkey_p")
            nc.vector.tensor_scalar(nkey_p, key_p, -1.0, None, op0=OP.mult)

            # alpha row form (via PE transpose) -> alpha output + key row
            alphat_ps = psmisc.tile([nch, P], f32, tag="misc")
            nc.tensor.transpose(alphat_ps, alpha_p, ident)
            alphat = small.tile([nch, P], f32, tag="alphat")
            nc.scalar.copy(alphat, alphat_ps)
            nc.gpsimd.dma_start(al_d[g].rearrange("(j q) -> j q", j=nch), alphat)
            inv9r = small.tile([nch, P], f32, tag="inv9r")
            nc.vector.tensor_scalar(inv9r, mr8, -9.0, 9.0, op0=OP.mult, op1=OP.add)
            keyt = small.tile([nch, P], f32, tag="keyt")
            nc.vector.tensor_add(keyt, alphat, inv9r)

            # key row -> DRAM scratch, broadcast-load to all 128 partitions
            nc.scalar.dma_start(ks_d[g].rearrange("(j q) -> j q", j=nch), keyt)
            k_b = kbp.tile([P, N], f32, tag="k_b")
            nc.gpsimd.dma_start(k_b, bass.AP(ks_d.tensor, g * N, [[0, P], [1, N]]))

            # rank via sign-count: acc[n] = sum_m sign(key[m] - key[n])
            rank = small.tile([P, nch], f32, tag="rank")
            for r in range(nch):
                s_t = signp.tile([P, N], bf16, tag="s_t")
                nc.scalar.activation(s_t, k_b, AF.Sign,
                                     bias=nkey_p[:, r : r + 1],
                                     accum_out=rank[:, r : r + 1])

            # keep = valid & (acc <= 1023 - 2*N_rm)
            keep_p = small.tile([P, nch], f32, tag="keep_p")
            nc.vector.tensor_scalar(keep_p, rank, thr_bc[:, g : g + 1], None,
                                    op0=OP.is_le)
            nc.vector.tensor_mul(keep_p, keep_p, mp)

            # N_pooled
            kc = small.tile([P, 1], f32, tag="kc")
            nc.vector.reduce_sum(kc, keep_p, axis=AX.X)
            npl_ps = psmisc.tile([1, 1], f32, tag="misc")
            nc.tensor.matmul(npl_ps, lhsT=ones_col, rhs=kc, start=True, stop=True)
            nc.vector.tensor_copy(npl_row[0:1, g : g + 1], npl_ps)

            # keep row form: mask_new output + bf16 row for outer products
            keept_ps = psmisc.tile([nch, P], f32, tag="misc")
            nc.tensor.transpose(keept_ps, keep_p, ident)
            keept_u8 = small.tile([nch, P], u8, tag="keept_u8")
            nc.vector.tensor_copy(keept_u8, keept_ps)
            nc.gpsimd.dma_start(mn_d[g].rearrange("(j q) -> j q", j=nch), keept_u8)
            keept_bf = small.tile([nch, P], bf16, tag="keept_bf")
            nc.scalar.copy(keept_bf, keept_ps)
            keep_row = rows.tile([1, N], bf16, tag="keep_row")
            nc.scalar.dma_start(keep_row, keept_bf)
            keep_rows.append(keep_row)

        nc.gpsimd.dma_start(npl_d, npl_row)

        # ---------------- phase 2: A masking -----------------------------
        for g in range(gpc):
            ar = a_d[g].rearrange("(q p) m -> p q m", p=P)
            aw = ao_d[g].rearrange("(q p) m -> p q m", p=P)
            kr = keep_rows[g]
            for qs in range(0, nch, QQ):
                a_t = apool.tile([P, QQ, N], f32, tag="a_t")
                nc.sync.dma_start(a_t, ar[:, qs : qs + QQ, :])
                for s in range(QQ):
                    q = qs + s
                    mm = psmm.tile([P, N], f32, tag="mm")
                    for h in range(2):
                        nc.tensor.matmul(
                            mm[:, h * 512 : (h + 1) * 512],
                            lhsT=kr[0:1, q * P : (q + 1) * P],
                            rhs=kr[0:1, h * 512 : (h + 1) * 512],
                            start=True, stop=True,
                        )
                    nc.vector.tensor_mul(a_t[:, s, :], a_t[:, s, :], mm)
                nc.gpsimd.dma_start(aw[:, qs : qs + QQ, :], a_t)

    nc.compile()
    return nc


_PROG = None


def _get_prog():
    global _PROG
    if _PROG is None:
        _PROG = build_program()
    return _PROG


def _ensure_ntff_hook():
    """The agent image's antenv lacks axon_hooks; synthesize it and install
    the ctypes NTFF profile hook so trace=True yields exec_time_ns."""
    import sys
    import types

    try:
        import antenv.axon_hooks  # noqa: F401
        return
    except ImportError:
        pass
    mod = types.ModuleType("antenv.axon_hooks")
    _hook = [None]
    mod.get_axon_ntff_profile_hook = lambda: _hook[0]
    mod.set_axon_ntff_profile_hook = lambda h: _hook.__setitem__(0, h)
    sys.modules["antenv.axon_hooks"] = mod
    try:
        import antenv
        antenv.axon_hooks = mod
    except ImportError:
        pass
    try:
        from trn_agent_boot.trn_boot import _ntff_profile_via_ctypes
        h = _ntff_profile_via_ctypes("/opt/axon/libaxon_pjrt.so")
        if h is not None:
            _hook[0] = h
    except Exception:
        pass


def prep_core_inputs(x, A, maskf, thr, c, gpc=GPC):
    s = slice(c * gpc, (c + 1) * gpc)
    mask_pc = maskf[s].reshape(gpc, NCH, P).transpose(0, 2, 1)
    return {
        "x_in": np.ascontiguousarray(x[s]),
        "a_in": np.ascontiguousarray(A[s]),
        "mask_p": np.ascontiguousarray(mask_pc),
        "mask_r": np.ascontiguousarray(maskf[s]),
        "thr": np.ascontiguousarray(thr[s]),
        "w": None,  # filled by caller
        "ident": None,
    }


def kernel(x, A, mask, N_nodes, proj_w, trace=False):
    x = np.asarray(x, dtype=np.float32)
    A = np.asarray(A, dtype=np.float32)
    mask = np.asarray(mask)
    N_nodes = np.asarray(N_nodes, dtype=np.int32)
    w = np.ascontiguousarray(np.asarray(proj_w, dtype=np.float32).reshape(C))

    maskf = mask.astype(np.float32)
    n_rm = np.round(
        N_nodes.astype(np.float32) * np.float32(1.0 - TOPK_RATIO)
    ).astype(np.int32)
    thr = (1023 - 2 * n_rm).astype(np.float32)
    ident = np.ascontiguousarray(np.eye(P, dtype=np.float32))

    nc = _get_prog()
    in_maps = []
    for c in range(NCORES):
        m = prep_core_inputs(x, A, maskf, thr, c)
        m["w"] = w
        m["ident"] = ident
        in_maps.append(m)

    if trace:
        _ensure_ntff_hook()
    res = bass_utils.run_bass_kernel_spmd(
        nc, in_maps, core_ids=list(range(NCORES)), trace=trace
    )
    outs = res.results

    x_out = np.concatenate([outs[c]["x_out"] for c in range(NCORES)], axis=0)
    a_out = np.concatenate([outs[c]["a_out"] for c in range(NCORES)], axis=0)
    mask_new = np.concatenate(
        [outs[c]["mask_new"] for c in range(NCORES)], axis=0
    ).astype(bool)
    alpha = np.concatenate([outs[c]["alpha_out"] for c in range(NCORES)], axis=0)
    n_pooled = np.concatenate(
        [outs[c]["n_pooled"] for c in range(NCORES)], axis=0
    ).astype(np.int32)

    if trace:
        kernel.last_exec_time_ns = res.exec_time_ns
        kernel.last_results = res
    return x_out, a_out, mask_new, alpha, n_pooled


# revision 21
# speedup vs baseline: 1.1522x; 1.1522x over previous
"""Trainium2 Bass kernel for nn_AttentionPooling (topk_masking).

Computation per graph b (B=32 graphs, batch-sharded over 8 NeuronCores):
  alpha_pre = clip(x @ w, +-60); alpha = exp(alpha_pre)*mask / (sum + eps)
  x_out = x * alpha[:, None]
  drop the N_remove = round(0.2*N_nodes) valid nodes with smallest alpha:
    rank[n] computed by sign-counting: acc[n] = sum_m sign(key[m]-key[n])
    (key = alpha for valid nodes, 9.0 for invalid) => acc = 1023 - 2*rank,
    keep <=> rank >= N_remove <=> acc <= 1023 - 2*N_remove  (and valid)
  A_out = A * outer(keep, keep);  mask_new = keep;  N_pooled = sum(keep)

Device mapping per core (4 graphs):
  VectorE: matvec (tensor_tensor_reduce), x scaling, A *= mm, small [128,8] ops
  ScalarE: exp (+row-accumulate), Sign compare tiles (vs key broadcast, with
           per-partition -key bias and free-dim accumulate = rank in one pass)
  TensorE: partition-sum reductions (ones-matmul), [128,8] transposes,
           keep outer-products (bf16, exact for 0/1 masks)
  DMA:     A/x streaming; key-row broadcast built by SBUF->SBUF gather+bcast
"""

import numpy as np

import concourse.bass as bass
import concourse.bacc as bacc
import concourse.mybir as mybir
import concourse.tile as tile
from concourse import bass_utils

B, N, C = 32, 1024, 256
P = 128
NCH = N // P              # 8 chunks of 128 nodes
NCORES = 8
GPC = B // NCORES         # graphs per core
QQ = 2                    # A chunks per DMA (2 -> 1MB tiles, 2MB loads)
TOPK_RATIO = 0.8
CLAMP = 60.0
EPS = 1e-7

f32 = mybir.dt.float32
bf16 = mybir.dt.bfloat16
u8 = mybir.dt.uint8
i32 = mybir.dt.int32
AF = mybir.ActivationFunctionType
OP = mybir.AluOpType
AX = mybir.AxisListType


def build_program(gpc=GPC, nch=NCH):
    from contextlib import ExitStack

    nc = bacc.Bacc("TRN2", target_bir_lowering=False, debug=False)

    x_d = nc.dram_tensor("x_in", [gpc, N, C], f32, kind="ExternalInput").ap()
    a_d = nc.dram_tensor("a_in", [gpc, N, N], f32, kind="ExternalInput").ap()
    mp_d = nc.dram_tensor("mask_p", [gpc, P, nch], f32, kind="ExternalInput").ap()
    mr_d = nc.dram_tensor("mask_r", [gpc, N], f32, kind="ExternalInput").ap()
    thr_d = nc.dram_tensor("thr", [gpc], f32, kind="ExternalInput").ap()
    w_d = nc.dram_tensor("w", [C], f32, kind="ExternalInput").ap()
    id_d = nc.dram_tensor("ident", [P, P], f32, kind="ExternalInput").ap()
    oh_d = nc.dram_tensor("onehot", [nch, nch * P], f32, kind="ExternalInput").ap()

    xo_d = nc.dram_tensor("x_out", [gpc, N, C], f32, kind="ExternalOutput").ap()
    ao_d = nc.dram_tensor("a_out", [gpc, N, N], f32, kind="ExternalOutput").ap()
    mn_d = nc.dram_tensor("mask_new", [gpc, N], u8, kind="ExternalOutput").ap()
    al_d = nc.dram_tensor("alpha_out", [gpc, N], f32, kind="ExternalOutput").ap()
    npl_d = nc.dram_tensor("n_pooled", [gpc], i32, kind="ExternalOutput").ap()

    with tile.TileContext(nc) as tc, ExitStack() as ctx:
        consts = ctx.enter_context(tc.tile_pool(name="consts", bufs=1))
        xpool = ctx.enter_context(tc.tile_pool(name="xpool", bufs=2))
        small = ctx.enter_context(tc.tile_pool(name="small", bufs=2))
        rows = ctx.enter_context(tc.tile_pool(name="rows", bufs=gpc))
        kbp = ctx.enter_context(tc.tile_pool(name="kbp", bufs=2))
        signp = ctx.enter_context(tc.tile_pool(name="signp", bufs=2))
        apool = ctx.enter_context(tc.tile_pool(name="apool", bufs=5))
        scratch = ctx.enter_context(tc.tile_pool(name="scratch", bufs=2))
        psmm = ctx.enter_context(tc.tile_pool(name="psmm", bufs=2, space="PSUM"))
        psmisc = ctx.enter_context(tc.tile_pool(name="psmisc", bufs=2, space="PSUM"))
        pskb = ctx.enter_context(tc.tile_pool(name="pskb", bufs=1, space="PSUM"))

        # constants
        w8 = consts.tile([P, C], f32)
        nc.gpsimd.dma_start(w8, bass.AP(w_d.tensor, 0, [[0, P], [1, C]]))
        ident = consts.tile([P, P], f32)
        nc.sync.dma_start(ident, id_d)
        thr_bc = consts.tile([P, gpc], f32)
        nc.gpsimd.dma_start(thr_bc, bass.AP(thr_d.tensor, 0, [[0, P], [1, gpc]]))
        ones_col = consts.tile([P, 1], f32)
        nc.vector.memset(ones_col, 1.0)
        ones_row = consts.tile([1, P], f32)
        nc.vector.memset(ones_row, 1.0)
        onehot = consts.tile([nch, nch * P], f32)
        nc.sync.dma_start(onehot, oh_d)
        npl_row = consts.tile([1, gpc], i32)

        keep_rows = []

        # ---------------- phase 1: per-graph alpha / top-k mask ----------
        for g in range(gpc):
            x_t = xpool.tile([P, nch, C], f32, tag="x_t")
            nc.sync.dma_start(x_t, x_d[g].rearrange("(j p) c -> p j c", p=P))
            mp = small.tile([P, nch], f32, tag="mp")
            nc.sync.dma_start(mp, mp_d[g])
            mr8 = small.tile([nch, P], f32, tag="mr8")
            nc.sync.dma_start(mr8, mr_d[g].rearrange("(j q) -> j q", j=nch))

            # matvec: pre[:, j] = sum_c x[:, j, c] * w[c]
            pre = small.tile([P, nch], f32, tag="pre")
            prod = scratch.tile([P, nch, C], f32, tag="prod")
            nc.vector.tensor_mul(
                prod, x_t, w8.unsqueeze(1).to_broadcast((P, nch, C))
            )
            nc.vector.tensor_reduce(pre, prod, axis=AX.X, op=OP.add)
            nc.vector.tensor_scalar(pre, pre, CLAMP, None, op0=OP.min)
            nc.vector.tensor_scalar(pre, pre, -CLAMP, None, op0=OP.max)

            # exp(pre - 1000*(1-mask)) -> exact 0 on invalid nodes
            shift = small.tile([P, nch], f32, tag="shift")
            nc.vector.tensor_scalar(shift, mp, 1000.0, 1000.0, op0=OP.mult,
                                    op1=OP.subtract)
            pre2 = small.tile([P, nch], f32, tag="pre2")
            nc.vector.tensor_add(pre2, pre, shift)
            e_t = small.tile([P, nch], f32, tag="e_t")
            esum = small.tile([P, 1], f32, tag="esum")
            nc.scalar.activation(e_t, pre2, AF.Exp, accum_out=esum)

            # S = sum(esum) over partitions; Sinv = 1/(S+eps)
            s_ps = psmisc.tile([1, 1], f32, tag="misc")
            nc.tensor.matmul(s_ps, lhsT=ones_col, rhs=esum, start=True, stop=True)
            seps = small.tile([1, 1], f32, tag="seps")
            nc.vector.tensor_scalar(seps, s_ps, EPS, None, op0=OP.add)
            sinv = small.tile([1, 1], f32, tag="sinv")
            nc.vector.reciprocal(sinv, seps)
            sinv_ps = psmisc.tile([P, 1], f32, tag="misc")
            nc.tensor.matmul(sinv_ps, lhsT=ones_row, rhs=sinv, start=True, stop=True)
            sinv_col = small.tile([P, 1], f32, tag="sinv_col")
            nc.scalar.copy(sinv_col, sinv_ps)

            alpha_p = small.tile([P, nch], f32, tag="alpha_p")
            nc.vector.tensor_scalar(alpha_p, e_t, sinv_col, None, op0=OP.mult)

            # x_out = x * alpha (in place), store
            for j in range(nch):
                nc.vector.tensor_scalar(x_t[:, j, :], x_t[:, j, :],
                                        alpha_p[:, j : j + 1], None, op0=OP.mult)
            nc.gpsimd.dma_start(xo_d[g].rearrange("(j p) c -> p j c", p=P), x_t)

            # key = alpha + 9*(1-mask); neg for use as activation bias
            inv9 = small.tile([P, nch], f32, tag="inv9")
            nc.vector.tensor_scalar(inv9, mp, -9.0, 9.0, op0=OP.mult, op1=OP.add)
            key_p = small.tile([P, nch], f32, tag="key_p")
            nc.vector.tensor_add(key_p, alpha_p, inv9)
            nkey_p = small.tile([P, nch], f32, tag="nkey_p")
            nc.vector.tensor_scalar(nkey_p, key_p, -1.0, None, op0=OP.mult)

            # alpha row form (via PE transpose) -> alpha output + key row
            alphat_ps = psmisc.tile([nch, P], f32, tag="misc")
            nc.tensor.transpose(alphat_ps, alpha_p, ident)
            alphat = small.tile([nch, P], f32, tag="alphat")
            nc.scalar.copy(alphat, alphat_ps)
            nc.gpsimd.dma_start(al_d[g].rearrange("(j q) -> j q", j=nch), alphat)
            inv9r = small.tile([nch, P], f32, tag="inv9r")
            nc.vector.tensor_scalar(inv9r, mr8, -9.0, 9.0, op0=OP.mult, op1=OP.add)
            keyt = small.tile([nch, P], f32, tag="keyt")
            nc.vector.tensor_add(keyt, alphat, inv9r)

            # broadcast key row to all 128 partitions on PE via one-hot
            # selection: K_b[:, j*128:(j+1)*128] = onehot_j.T @ keyt  (K=8)
            k_b = pskb.tile([P, N], f32, tag="k_b")
            for j in range(nch):
                nc.tensor.matmul(
                    k_b[:, j * P : (j + 1) * P],
                    lhsT=onehot[:, j * P : (j + 1) * P],
                    rhs=keyt,
                    start=True, stop=True,
                )

            # rank via sign-count: acc[n] = sum_m sign(key[m] - key[n])
            rank = small.tile([P, nch], f32, tag="rank")
            for r in range(nch):
                s_t = signp.tile([P, N], bf16, tag="s_t")
                nc.scalar.activation(s_t, k_b, AF.Sign,
                                     bias=nkey_p[:, r : r + 1],
                                     accum_out=rank[:, r : r + 1])

            # keep = valid & (acc <= 1023 - 2*N_rm)
            keep_p = small.tile([P, nch], f32, tag="keep_p")
            nc.vector.tensor_scalar(keep_p, rank, thr_bc[:, g : g + 1], None,
                                    op0=OP.is_le)
            nc.vector.tensor_mul(keep_p, keep_p, mp)

            # N_pooled
            kc = small.tile([P, 1], f32, tag="kc")
            nc.vector.reduce_sum(kc, keep_p, axis=AX.X)
            npl_ps = psmisc.tile([1, 1], f32, tag="misc")
            nc.tensor.matmul(npl_ps, lhsT=ones_col, rhs=kc, start=True, stop=True)
            nc.vector.tensor_copy(npl_row[0:1, g : g + 1], npl_ps)

            # keep row form: mask_new output + bf16 row for outer products
            keept_ps = psmisc.tile([nch, P], f32, tag="misc")
            nc.tensor.transpose(keept_ps, keep_p, ident)
            keept_u8 = small.tile([nch, P], u8, tag="keept_u8")
            nc.vector.tensor_copy(keept_u8, keept_ps)
            nc.gpsimd.dma_start(mn_d[g].rearrange("(j q) -> j q", j=nch), keept_u8)
            keept_bf = small.tile([nch, P], bf16, tag="keept_bf")
            nc.scalar.copy(keept_bf, keept_ps)
            keep_row = rows.tile([1, N], bf16, tag="keep_row")
            nc.scalar.dma_start(keep_row, keept_bf)
            keep_rows.append(keep_row)

        nc.gpsimd.dma_start(npl_d, npl_row)

        # ---------------- phase 2: A masking -----------------------------
        for g in range(gpc):
            ar = a_d[g].rearrange("(q p) m -> p q m", p=P)
            aw = ao_d[g].rearrange("(q p) m -> p q m", p=P)
            kr = keep_rows[g]
            for qs in range(0, nch, QQ):
                a_t = apool.tile([P, QQ, N], f32, tag="a_t")
                nc.sync.dma_start(a_t, ar[:, qs : qs + QQ, :])
                for s in range(QQ):
                    q = qs + s
                    mm = psmm.tile([P, N], f32, tag="mm")
                    for h in range(2):
                        nc.tensor.matmul(
                            mm[:, h * 512 : (h + 1) * 512],
                            lhsT=kr[0:1, q * P : (q + 1) * P],
                            rhs=kr[0:1, h * 512 : (h + 1) * 512],
                            start=True, stop=True,
                        )
                    nc.vector.tensor_mul(a_t[:, s, :], a_t[:, s, :], mm)
                nc.gpsimd.dma_start(aw[:, qs : qs + QQ, :], a_t)

    nc.compile()
    return nc


_PROG = None


def _get_prog():
    global _PROG
    if _PROG is None:
        _PROG = build_program()
    return _PROG


def _ensure_ntff_hook():
    """The agent image's antenv lacks axon_hooks; synthesize it and install
    the ctypes NTFF profile hook so trace=True yields exec_time_ns."""
    import sys
    import types

    try:
        import antenv.axon_hooks  # noqa: F401
        return
    except ImportError:
        pass
    mod = types.ModuleType("antenv.axon_hooks")
    _hook = [None]
    mod.get_axon_ntff_profile_hook = lambda: _hook[0]
    mod.set_axon_ntff_profile_hook = lambda h: _hook.__setitem__(0, h)
    sys.modules["antenv.axon_hooks"] = mod
    try:
        import antenv
        antenv.axon_hooks = mod
    except ImportError:
        pass
    try:
        from trn_agent_boot.trn_boot import _ntff_profile_via_ctypes
        h = _ntff_profile_via_ctypes("/opt/axon/libaxon_pjrt.so")
        if h is not None:
            _hook[0] = h
    except Exception:
        pass


def make_onehot():
    oh = np.zeros((NCH, NCH * P), dtype=np.float32)
    for j in range(NCH):
        oh[j, j * P : (j + 1) * P] = 1.0
    return np.ascontiguousarray(oh)


def prep_core_inputs(x, A, maskf, thr, c, gpc=GPC):
    s = slice(c * gpc, (c + 1) * gpc)
    mask_pc = maskf[s].reshape(gpc, NCH, P).transpose(0, 2, 1)
    return {
        "x_in": np.ascontiguousarray(x[s]),
        "a_in": np.ascontiguousarray(A[s]),
        "mask_p": np.ascontiguousarray(mask_pc),
        "mask_r": np.ascontiguousarray(maskf[s]),
        "thr": np.ascontiguousarray(thr[s]),
        "w": None,  # filled by caller
        "ident": None,
    }


def kernel(x, A, mask, N_nodes, proj_w, trace=False):
    x = np.asarray(x, dtype=np.float32)
    A = np.asarray(A, dtype=np.float32)
    mask = np.asarray(mask)
    N_nodes = np.asarray(N_nodes, dtype=np.int32)
    w = np.ascontiguousarray(np.asarray(proj_w, dtype=np.float32).reshape(C))

    maskf = mask.astype(np.float32)
    n_rm = np.round(
        N_nodes.astype(np.float32) * np.float32(1.0 - TOPK_RATIO)
    ).astype(np.int32)
    thr = (1023 - 2 * n_rm).astype(np.float32)
    ident = np.ascontiguousarray(np.eye(P, dtype=np.float32))
    onehot = make_onehot()

    nc = _get_prog()
    in_maps = []
    for c in range(NCORES):
        m = prep_core_inputs(x, A, maskf, thr, c)
        m["w"] = w
        m["ident"] = ident
        m["onehot"] = onehot
        in_maps.append(m)

    if trace:
        _ensure_ntff_hook()
    res = bass_utils.run_bass_kernel_spmd(
        nc, in_maps, core_ids=list(range(NCORES)), trace=trace
    )
    outs = res.results

    x_out = np.concatenate([outs[c]["x_out"] for c in range(NCORES)], axis=0)
    a_out = np.concatenate([outs[c]["a_out"] for c in range(NCORES)], axis=0)
    mask_new = np.concatenate(
        [outs[c]["mask_new"] for c in range(NCORES)], axis=0
    ).astype(bool)
    alpha = np.concatenate([outs[c]["alpha_out"] for c in range(NCORES)], axis=0)
    n_pooled = np.concatenate(
        [outs[c]["n_pooled"] for c in range(NCORES)], axis=0
    ).astype(np.int32)

    if trace:
        kernel.last_exec_time_ns = res.exec_time_ns
        kernel.last_results = res
    return x_out, a_out, mask_new, alpha, n_pooled
